# revision 21
# baseline (speedup 1.0000x reference)
"""ArcFace head forward on 8 Trainium2 NeuronCores (Bass, raw blocks).

Math (per batch row b, class c in {0,1}):
    feat_n = feat / max(||feat||, 1e-12)
    W_n    = W / max(||W_row||, 1e-12)
    cos    = clip(feat_n . W_n[c], -1+1e-7, 1-1e-7)
    cos_m  = cos*cos(0.5) - sqrt(1-cos^2)*sin(0.5)   # == cos(arccos(cos)+0.5)
    out    = 64 * (cos if c != label[b] else cos_m)

Distribution: pure data parallel: feat/label sharded along batch over 8
cores, W replicated; forward only, so no collectives.

Per-core pipeline (shard = 16384 rows x 512 f32 = 32 MB):
  - GpSimd SWDGE streams feat in 16 blocks of [128, 4096], converting
    f32 -> bf16 in flight (SDMA cast); 16 KB-contiguous per partition
  - TensorE: per [128,512] sub-tile, 4x PE-transpose (128x128 bf16) into
    PSUM, then 4 accumulating matmuls (feat^T chunk stationary,
    normalized-W^T chunk moving) -> both class dots in PSUM [128,2]
  - ScalarE: copies transposed tiles PSUM->SBUF (matmul stationary must
    come from SBUF) + a share of the row sum-of-squares
    (activation Square + accum_out)
  - VectorE: the other share of sum-of-squares (fused self-mult STT),
    per-block dot harvest from PSUM, and the batched epilogue
    (norms, clip, margin identity, one-hot blend, scale by 64)
Row mapping: batch row b = blk*1024 + p*8 + j lives on partition p,
accumulator column t = blk*8 + j. Host glue only shards/reorders.
"""

import sys
from contextlib import ExitStack

import numpy as np

for _p in ("/opt/trn_rl_repo",):
    if _p not in sys.path:
        sys.path.insert(0, _p)

import concourse.bass as bass
import concourse.mybir as mybir
from concourse.bass_utils import run_bass_kernel_spmd

B, D, C = 131072, 512, 2
NCORES = 8
BS = B // NCORES          # 16384 rows per core
SUB = 512                 # columns per compute sub-tile
SPB = 8                   # sub-tiles per block
BLK_COLS = SUB * SPB      # 4096 (1024 batch rows)
NBLK = BS // (128 * SPB)  # 16
T = BS // 128             # 128 accumulator columns
NB = 4                    # fb ring depth (1 MB bf16 each)
SQ_DVE = 7                # sub-tiles per block whose sumsq runs on VectorE
PAIRS = T // 2            # transpose/copy granularity: 2 sub-tiles per pair
PPB = SPB // 2            # pairs per block (4)

S_SCALE = 64.0
MARGIN = 0.5
EPS = 1e-7
NORM_EPS = 1e-12
COS_M = float(np.cos(MARGIN))
SIN_M = float(np.sin(MARGIN))

F32 = mybir.dt.float32
BF16 = mybir.dt.bfloat16


def build_nc():
    nc = bass.Bass()
    AF = mybir.ActivationFunctionType
    OP = mybir.AluOpType

    feat = nc.declare_dram_parameter("feat", [BS, D], F32, isOutput=False)
    wrep = nc.declare_dram_parameter("wrep", [128, C * D], F32, isOutput=False)
    wrepT = nc.declare_dram_parameter("wrepT", [128, 2 * 4], F32, isOutput=False)
    lab = nc.declare_dram_parameter("lab", [128, T], F32, isOutput=False)
    identf = nc.declare_dram_parameter("identf", [128, 128], F32, isOutput=False)
    out = nc.declare_dram_parameter("out", [128, C * T], F32, isOutput=True)

    # feat[blk*1024 + p*8 + j, d] -> view[blk, p, j*512+d] (16KB/partition)
    feat_v = feat[:].rearrange("(blk p j) d -> blk p (j d)", p=128, j=SPB)

    with ExitStack() as ctx:
        def sb(name, shape, dt):
            return ctx.enter_context(nc.sbuf_tensor(name, shape, dt))

        def psum(name, shape, dt):
            return ctx.enter_context(nc.psum_tensor(name, shape, dt))

        def sem(name):
            return ctx.enter_context(nc.semaphore(name))

        w_f = sb("w_f", [128, C * D], F32)
        wrepT_sb = sb("wrepT_sb", [128, 8], F32)
        w_bfT = sb("w_bfT", [128, 8], BF16)     # [p, c*4+k] = WnT chunk layout
        lab_t = sb("lab_t", [128, T], F32)
        identf_sb = sb("identf_sb", [128, 128], F32)
        ident_bf = sb("ident_bf", [128, 128], BF16)
        ss = sb("ss", [128, T], F32)
        dd = sb("dd", [128, C * T], F32)        # d0 | d1
        fbs = [sb(f"fb{k}", [128, BLK_COLS], BF16) for k in range(NB)]
        ftT = [sb(f"ftT{k}", [128, 2 * SUB], BF16) for k in range(2)]
        sq_scr = sb("sq_scr", [128, SUB], F32)
        tt_scr = sb("tt_scr", [128, SUB], BF16)
        wss = sb("wss", [128, C], F32)
        wnorm = sb("wnorm", [128, C], F32)
        winv = sb("winv", [128, C], F32)
        norm = sb("norm", [128, T], F32)
        inv = sb("inv", [128, T], F32)
        cos0 = sb("cos0", [128, T], F32)
        cos1 = sb("cos1", [128, T], F32)
        sq2 = sb("sq2", [128, T], F32)
        sin0 = sb("sin0", [128, T], F32)
        sin1 = sb("sin1", [128, T], F32)
        tmp1 = sb("tmp1", [128, T], F32)
        tmp2 = sb("tmp2", [128, T], F32)
        oh0 = sb("oh0", [128, T], F32)
        out_stage = sb("out_stage", [128, C * T], F32)

        tpb = [psum(f"tpb{k}", [128, 2 * SUB], BF16) for k in range(3)]
        DPR = NB + 1  # d_ps ring: PE block q is gated only through vdone(q-NB+1),
        # which implies dcopy(q-NB-1) done -> ring must exceed NB
        d_ps = [psum(f"dps{k}", [128, 2 * SPB], F32) for k in range(DPR)]

        pre = sem("pre")       # preamble DMAs
        ld = sem("ld")         # feat block loads (16 per block)
        petr = sem("petr")     # PE transposes done for sub-tile t -> t+1
        actcopy = sem("actcopy")  # ACT copy of sub-tile t done -> t+1
        pedot = sem("pedot")   # PE dots done for sub-tile t -> t+1
        ablk = sem("ablk")     # ACT done with block i -> i+1
        vdone = sem("vdone")   # DVE done with block i -> i+1 (gates fb reuse)
        sepi = sem("sepi")     # scalar milestones
        vepi = sem("vepi")     # vector milestones
        outd = sem("outd")

        # strided view of w_bfT: [p, k, c] with c-stride 4
        w_bfT_k = w_bfT[:].rearrange("p (c k) -> p k c", k=4)

        with nc.Block() as block:

            @block.sync
            def _(sync):
                sync.dma_start(out=w_f[:], in_=wrep[:]).then_inc(pre, 16)
                sync.dma_start(out=wrepT_sb[:], in_=wrepT[:]).then_inc(pre, 16)
                sync.dma_start(out=lab_t[:], in_=lab[:]).then_inc(pre, 16)
                sync.dma_start(out=identf_sb[:], in_=identf[:]).then_inc(pre, 16)
                sync.wait_ge(vepi, 3)
                sync.dma_start(out=out[:], in_=out_stage[:]).then_inc(outd, 16)
                sync.wait_ge(outd, 16)

            @block.gpsimd
            def _(gpsimd):
                for i in range(NBLK):
                    if i >= NB:
                        gpsimd.wait_ge(vdone, i - NB + 1)
                    # converting DMA: f32 DRAM -> bf16 SBUF
                    gpsimd.dma_start(
                        out=fbs[i % NB][:], in_=feat_v[i]
                    ).then_inc(ld, 16)

            @block.tensor
            def _(tensor):
                tensor.wait_ge(vepi, 1)  # ident_bf ready
                # software-pipelined: transposes run one pair ahead of dots
                for u in range(PAIRS + 1):
                    if u < PAIRS:
                        i = u // PPB
                        if u % PPB == 0:
                            tensor.wait_ge(ld, 16 * (i + 1))
                        fb = fbs[i % NB]
                        for s in range(2):
                            j = (u % PPB) * 2 + s
                            for k in range(4):
                                ins = tensor.transpose(
                                    tpb[u % 3][:, s * SUB + k * 128:
                                               s * SUB + (k + 1) * 128],
                                    fb[:, j * SUB + k * 128:
                                       j * SUB + (k + 1) * 128],
                                    ident_bf[:],
                                )
                        ins.then_inc(petr, 1)
                    if u >= 1:
                        ud = u - 1
                        tensor.wait_ge(actcopy, ud + 1)
                        for s in range(2):
                            td = 2 * ud + s
                            i_d, j_d = td // SPB, td % SPB
                            for k in range(4):
                                ins = tensor.matmul(
                                    d_ps[i_d % DPR][:, 2 * j_d:2 * j_d + 2],
                                    ftT[ud % 2][:, s * SUB + k * 128:
                                                s * SUB + (k + 1) * 128],
                                    w_bfT_k[:, k, :],
                                    start=(k == 0), stop=(k == 3),
                                )
                            ins.then_inc(pedot, 1)

            @block.scalar
            def _(scalar):
                # --- W norms ---
                scalar.wait_ge(pre, 64)
                for c in range(C):
                    scalar.activation(
                        out=sq_scr[:], in_=w_f[:, c * D:(c + 1) * D],
                        func=AF.Square, accum_out=wss[:, c:c + 1],
                    )
                scalar.activation(out=wnorm[:], in_=wss[:], func=AF.Sqrt).then_inc(
                    sepi, 1
                )  # sepi=1
                scalar.wait_ge(vepi, 1)  # winv ready
                for c in range(C):
                    # w_bfT = wrepT * (1/||W_c||), cast to bf16.
                    # (tensor_scalar with an AP scalar = TensorScalarPtr
                    # mis-reads the scalar on this stack; Copy-with-scale on
                    # ScalarE is the validated path.)
                    scalar.activation(
                        out=w_bfT[:, c * 4:(c + 1) * 4],
                        in_=wrepT_sb[:, c * 4:(c + 1) * 4],
                        func=AF.Copy, scale=winv[:, c:c + 1],
                    )

                # --- main loop: PSUM->SBUF copies + sumsq share ---
                for i in range(NBLK):
                    for up in range(PPB):
                        u = i * PPB + up
                        scalar.wait_ge(petr, u + 1)
                        scalar.activation(
                            out=ftT[u % 2][:], in_=tpb[u % 3][:], func=AF.Copy
                        ).then_inc(actcopy, 1)
                    fb = fbs[i % NB]
                    for j in range(SQ_DVE, SPB):
                        t = i * SPB + j
                        ins = scalar.activation(
                            out=sq_scr[:], in_=fb[:, j * SUB:(j + 1) * SUB],
                            func=AF.Square, accum_out=ss[:, t:t + 1],
                        )
                    ins.then_inc(ablk, 1)

                # --- epilogue (scalar part) ---
                scalar.wait_ge(vdone, NBLK)  # DVE sumsq columns all written
                scalar.activation(out=norm[:], in_=ss[:], func=AF.Sqrt).then_inc(
                    sepi, 1
                )  # sepi=2
                scalar.wait_ge(vepi, 2)  # cos0/cos1 ready
                scalar.activation(out=sq2[:], in_=cos0[:], func=AF.Square)
                scalar.activation(
                    out=sin0[:], in_=sq2[:], func=AF.Sqrt, bias=1.0, scale=-1.0
                )
                scalar.activation(out=sq2[:], in_=cos1[:], func=AF.Square)
                scalar.activation(
                    out=sin1[:], in_=sq2[:], func=AF.Sqrt, bias=1.0, scale=-1.0
                ).then_inc(sepi, 1)  # sepi=3

            @block.vector
            def _(vector):
                # --- preamble: winv + identity cast ---
                vector.wait_ge(sepi, 1)
                vector.tensor_scalar(wnorm[:], wnorm[:], NORM_EPS, None, OP.max)
                vector.reciprocal(winv[:], wnorm[:])
                vector.tensor_copy(ident_bf[:], identf_sb[:]).then_inc(vepi, 1)

                # --- main loop: dot harvest + sumsq share ---
                for i in range(NBLK):
                    if i >= 1:
                        vector.wait_ge(pedot, SPB * i)
                        vector.tensor_copy(
                            dd[:].rearrange("p (c t) -> p c t", c=2)
                                [:, :, SPB * (i - 1):SPB * i],
                            d_ps[(i - 1) % DPR][:].rearrange(
                                "p (j c) -> p c j", c=2),
                        )
                    fb = fbs[i % NB]
                    vector.wait_ge(ablk, i + 1)
                    for j in range(SQ_DVE):
                        t = i * SPB + j
                        sl = slice(j * SUB, (j + 1) * SUB)
                        ins = vector.scalar_tensor_tensor(
                            out=tt_scr[:], in0=fb[:, sl], scalar=1.0,
                            in1=fb[:, sl], op0=OP.mult, op1=OP.mult,
                            accum_out=ss[:, t:t + 1],
                        )
                    ins.then_inc(vdone, 1)
                vector.wait_ge(pedot, SPB * NBLK)
                vector.tensor_copy(
                    dd[:].rearrange("p (c t) -> p c t", c=2)
                        [:, :, SPB * (NBLK - 1):SPB * NBLK],
                    d_ps[(NBLK - 1) % DPR][:].rearrange("p (j c) -> p c j", c=2),
                )

                # --- epilogue (vector part) ---
                vector.wait_ge(sepi, 2)  # norm ready
                vector.tensor_scalar(norm[:], norm[:], NORM_EPS, None, OP.max)
                vector.reciprocal(inv[:], norm[:])
                vector.tensor_tensor(cos0[:], dd[:, 0:T], inv[:], OP.mult)
                vector.tensor_scalar(
                    cos0[:], cos0[:], 1.0 - EPS, -1.0 + EPS, OP.min, OP.max
                )
                vector.tensor_tensor(cos1[:], dd[:, T:2 * T], inv[:], OP.mult)
                vector.tensor_scalar(
                    cos1[:], cos1[:], 1.0 - EPS, -1.0 + EPS, OP.min, OP.max
                ).then_inc(vepi, 1)  # vepi=2

                vector.wait_ge(sepi, 3)  # sin0/sin1 ready
                for c, (cosv, sinv) in enumerate(((cos0, sin0), (cos1, sin1))):
                    vector.tensor_scalar(tmp1[:], cosv[:], COS_M, None, OP.mult)
                    vector.tensor_scalar(tmp2[:], sinv[:], SIN_M, None, OP.mult)
                    vector.tensor_tensor(tmp1[:], tmp1[:], tmp2[:], OP.subtract)
                    vector.tensor_tensor(tmp2[:], tmp1[:], cosv[:], OP.subtract)
                    if c == 0:
                        vector.tensor_scalar(
                            oh0[:], lab_t[:], -1.0, 1.0, OP.mult, OP.add
                        )
                        oh = oh0
                    else:
                        oh = lab_t
                    vector.tensor_tensor(tmp2[:], tmp2[:], oh[:], OP.mult)
                    vector.tensor_tensor(tmp2[:], cosv[:], tmp2[:], OP.add)
                    ins = vector.tensor_scalar(
                        out_stage[:, c * T:(c + 1) * T], tmp2[:],
                        S_SCALE, None, OP.mult,
                    )
                ins.then_inc(vepi, 1)  # vepi=3

    return nc


_NC = None


def _get_nc():
    global _NC
    if _NC is None:
        _NC = build_nc()
    return _NC


def _make_in_maps(feat, W, label):
    feat = np.ascontiguousarray(np.asarray(feat, dtype=np.float32))
    W = np.ascontiguousarray(np.asarray(W, dtype=np.float32))
    label = np.asarray(label)
    wr = np.ascontiguousarray(np.tile(W.reshape(1, C * D), (128, 1)))
    # wrepT[p, c*4+k] = W[c, k*128+p]
    wrT = np.ascontiguousarray(
        W.reshape(C, 4, 128).transpose(2, 0, 1).reshape(128, 8)
    )
    ident = np.eye(128, dtype=np.float32)
    in_maps = []
    for core in range(NCORES):
        fs = feat[core * BS:(core + 1) * BS]
        ls = label[core * BS:(core + 1) * BS].astype(np.float32)
        # lab_dev[p, blk*8+j] = label[blk*1024 + p*8 + j]
        ls = ls.reshape(NBLK, 128, SPB).transpose(1, 0, 2).reshape(128, T)
        in_maps.append(
            {"feat": np.ascontiguousarray(fs), "wrep": wr, "wrepT": wrT,
             "lab": np.ascontiguousarray(ls), "identf": ident}
        )
    return in_maps


def _assemble(results):
    outs = []
    for core in range(NCORES):
        o = np.asarray(results[core]["out"])       # [128, C*T]
        o = o.reshape(128, C, NBLK, SPB)            # [p, c, blk, j]
        o = o.transpose(2, 0, 3, 1).reshape(BS, C)  # [blk, p, j, c]
        outs.append(o)
    return np.concatenate(outs, axis=0)


def run(feat, W, label, trace=False, **kw):
    nc = _get_nc()
    in_maps = _make_in_maps(feat, W, label)
    res = run_bass_kernel_spmd(
        nc, in_maps, core_ids=list(range(NCORES)), trace=trace, **kw
    )
    return _assemble(res.results), res


def kernel(feat, W, label):
    out, _ = run(feat, W, label, trace=False)
    return out


# revision 23
# speedup vs baseline: 1.1132x; 1.1132x over previous
"""ArcFace head forward on 8 Trainium2 NeuronCores (Bass, raw blocks).

Math (per batch row b, class c in {0,1}):
    feat_n = feat / max(||feat||, 1e-12)
    W_n    = W / max(||W_row||, 1e-12)
    cos    = clip(feat_n . W_n[c], -1+1e-7, 1-1e-7)
    cos_m  = cos*cos(0.5) - sqrt(1-cos^2)*sin(0.5)   # == cos(arccos(cos)+0.5)
    out    = 64 * (cos if c != label[b] else cos_m)

Distribution: pure data parallel: feat/label sharded along batch over 8
cores, W replicated; forward only, so no collectives.

Per-core pipeline (shard = 16384 rows x 512 f32 = 32 MB):
  - GpSimd SWDGE streams feat in 16 blocks of [128, 4096], converting
    f32 -> bf16 in flight (SDMA cast); 16 KB-contiguous per partition
  - TensorE: per [128,512] sub-tile, 4x PE-transpose (128x128 bf16) into
    PSUM, then 4 accumulating matmuls (feat^T chunk stationary,
    normalized-W^T chunk moving) -> both class dots in PSUM [128,2]
  - ScalarE: copies transposed tiles PSUM->SBUF (matmul stationary must
    come from SBUF) + a share of the row sum-of-squares
    (activation Square + accum_out)
  - VectorE: the other share of sum-of-squares (fused self-mult STT),
    per-block dot harvest from PSUM, and the batched epilogue
    (norms, clip, margin identity, one-hot blend, scale by 64)
Row mapping: batch row b = blk*1024 + p*8 + j lives on partition p,
accumulator column t = blk*8 + j. Host glue only shards/reorders.
"""

import sys
from contextlib import ExitStack

import numpy as np

for _p in ("/opt/trn_rl_repo",):
    if _p not in sys.path:
        sys.path.insert(0, _p)

import concourse.bass as bass
import concourse.mybir as mybir
from concourse.bass_utils import run_bass_kernel_spmd

B, D, C = 131072, 512, 2
NCORES = 8
BS = B // NCORES          # 16384 rows per core
SUB = 512                 # columns per compute sub-tile
SPB = 8                   # sub-tiles per block
BLK_COLS = SUB * SPB      # 4096 (1024 batch rows)
NBLK = BS // (128 * SPB)  # 16
T = BS // 128             # 128 accumulator columns
NB = 4                    # fb ring depth (1 MB bf16 each)
SQ_DVE = 7                # sub-tiles per block whose sumsq runs on VectorE
PAIRS = T // 2            # transpose/copy granularity: 2 sub-tiles per pair
PPB = SPB // 2            # pairs per block (4)

S_SCALE = 64.0
MARGIN = 0.5
EPS = 1e-7
NORM_EPS = 1e-12
COS_M = float(np.cos(MARGIN))
SIN_M = float(np.sin(MARGIN))

F32 = mybir.dt.float32
BF16 = mybir.dt.bfloat16


def build_nc():
    nc = bass.Bass()
    AF = mybir.ActivationFunctionType
    OP = mybir.AluOpType

    feat = nc.declare_dram_parameter("feat", [BS, D], F32, isOutput=False)
    wrep = nc.declare_dram_parameter("wrep", [128, C * D], F32, isOutput=False)
    wrepT = nc.declare_dram_parameter("wrepT", [128, 2 * 4], F32, isOutput=False)
    lab = nc.declare_dram_parameter("lab", [128, T], F32, isOutput=False)
    identf = nc.declare_dram_parameter("identf", [128, 128], F32, isOutput=False)
    out = nc.declare_dram_parameter("out", [128, C * T], F32, isOutput=True)

    # feat[blk*1024 + p*8 + j, d] -> view[blk, p, j*512+d] (16KB/partition)
    feat_v = feat[:].rearrange("(blk p j) d -> blk p (j d)", p=128, j=SPB)

    with ExitStack() as ctx:
        def sb(name, shape, dt):
            return ctx.enter_context(nc.sbuf_tensor(name, shape, dt))

        def psum(name, shape, dt):
            return ctx.enter_context(nc.psum_tensor(name, shape, dt))

        def sem(name):
            return ctx.enter_context(nc.semaphore(name))

        w_f = sb("w_f", [128, C * D], F32)
        wrepT_sb = sb("wrepT_sb", [128, 8], F32)
        w_bfT = sb("w_bfT", [128, 8], BF16)     # [p, c*4+k] = WnT chunk layout
        lab_t = sb("lab_t", [128, T], F32)
        identf_sb = sb("identf_sb", [128, 128], F32)
        ss = sb("ss", [128, T], F32)
        dd = sb("dd", [128, C * T], F32)        # d0 | d1
        fbs = [sb(f"fb{k}", [128, BLK_COLS], BF16) for k in range(NB)]
        ftT = [sb(f"ftT{k}", [128, SUB], F32) for k in range(2)]
        sq_scr = sb("sq_scr", [128, SUB], F32)
        tt_scr = sb("tt_scr", [128, SUB], BF16)
        wss = sb("wss", [128, C], F32)
        wnorm = sb("wnorm", [128, C], F32)
        winv = sb("winv", [128, C], F32)
        norm = sb("norm", [128, T], F32)
        inv = sb("inv", [128, T], F32)
        cos0 = sb("cos0", [128, T], F32)
        cos1 = sb("cos1", [128, T], F32)
        sq2 = sb("sq2", [128, T], F32)
        sin0 = sb("sin0", [128, T], F32)
        sin1 = sb("sin1", [128, T], F32)
        tmp1 = sb("tmp1", [128, T], F32)
        tmp2 = sb("tmp2", [128, T], F32)
        oh0 = sb("oh0", [128, T], F32)
        out_stage = sb("out_stage", [128, C * T], F32)

        tpb = [psum(f"tpb{k}", [128, SUB], F32) for k in range(3)]
        DPR = NB + 1  # d_ps ring: PE block q is gated only through vdone(q-NB+1),
        # which implies dcopy(q-NB-1) done -> ring must exceed NB
        d_ps = [psum(f"dps{k}", [128, 2 * SPB], F32) for k in range(DPR)]

        pre = sem("pre")       # preamble DMAs
        ld = sem("ld")         # feat block loads (16 per block)
        petr = sem("petr")     # PE transposes done for sub-tile t -> t+1
        actcopy = sem("actcopy")  # ACT copy of sub-tile t done -> t+1
        pedot = sem("pedot")   # PE dots done for sub-tile t -> t+1
        ablk = sem("ablk")     # ACT done with block i -> i+1
        vdone = sem("vdone")   # DVE done with block i -> i+1 (gates fb reuse)
        sepi = sem("sepi")     # scalar milestones
        vepi = sem("vepi")     # vector milestones
        outd = sem("outd")

        # strided view of w_bfT: [p, g, c] with c-stride 4; g = qp*2+par
        w_bfT_g = w_bfT[:].rearrange("p (c g) -> p g c", g=4)

        with nc.Block() as block:

            @block.sync
            def _(sync):
                sync.dma_start(out=w_f[:], in_=wrep[:]).then_inc(pre, 16)
                sync.dma_start(out=wrepT_sb[:], in_=wrepT[:]).then_inc(pre, 16)
                sync.dma_start(out=lab_t[:], in_=lab[:]).then_inc(pre, 16)
                sync.dma_start(out=identf_sb[:], in_=identf[:]).then_inc(pre, 16)
                sync.wait_ge(vepi, 3)
                sync.dma_start(out=out[:], in_=out_stage[:]).then_inc(outd, 16)
                sync.wait_ge(outd, 16)

            @block.gpsimd
            def _(gpsimd):
                for i in range(NBLK):
                    if i >= NB:
                        gpsimd.wait_ge(vdone, i - NB + 1)
                    # converting DMA: f32 DRAM -> bf16 SBUF
                    gpsimd.dma_start(
                        out=fbs[i % NB][:], in_=feat_v[i]
                    ).then_inc(ld, 16)

            @block.tensor
            def _(tensor):
                tensor.wait_ge(pre, 64)  # identf_sb ready
                # software-pipelined: transposes run one pair ahead of dots.
                # Transposes operate on the f32 view of fb (bf16 pairs packed
                # per element): 4 transposes move a whole pair (2 sub-tiles).
                for u in range(PAIRS + 1):
                    if u < PAIRS:
                        i = u // PPB
                        if u % PPB == 0:
                            tensor.wait_ge(ld, 16 * (i + 1))
                        fb32 = fbs[i % NB][:].bitcast(F32)  # [128, 2048]
                        base = (u % PPB) * SUB              # f32 cols per pair
                        for q in range(4):
                            ins = tensor.transpose(
                                tpb[u % 3][:, q * 128:(q + 1) * 128],
                                fb32[:, base + q * 128: base + (q + 1) * 128],
                                identf_sb[:],
                            )
                        ins.then_inc(petr, 1)
                    if u >= 1:
                        ud = u - 1
                        tensor.wait_ge(actcopy, ud + 1)
                        # bf16 view of the packed transposed pair:
                        # [p, q, b, par] with q = s*2+m, D = 256*m + 2*dp + par
                        vw = ftT[ud % 2][:].bitcast(BF16).rearrange(
                            "p (q b par) -> p q par b", q=4, par=2)
                        for s in range(2):
                            td = 2 * ud + s
                            i_d, j_d = td // SPB, td % SPB
                            for g in range(4):
                                m, par = g // 2, g % 2
                                ins = tensor.matmul(
                                    d_ps[i_d % DPR][:, 2 * j_d:2 * j_d + 2],
                                    vw[:, s * 2 + m, par, :],
                                    w_bfT_g[:, g, :],
                                    start=(g == 0), stop=(g == 3),
                                )
                            ins.then_inc(pedot, 1)

            @block.scalar
            def _(scalar):
                # --- W norms ---
                scalar.wait_ge(pre, 64)
                for c in range(C):
                    scalar.activation(
                        out=sq_scr[:], in_=w_f[:, c * D:(c + 1) * D],
                        func=AF.Square, accum_out=wss[:, c:c + 1],
                    )
                scalar.activation(out=wnorm[:], in_=wss[:], func=AF.Sqrt).then_inc(
                    sepi, 1
                )  # sepi=1
                scalar.wait_ge(vepi, 1)  # winv ready
                for c in range(C):
                    # w_bfT = wrepT * (1/||W_c||), cast to bf16.
                    # (tensor_scalar with an AP scalar = TensorScalarPtr
                    # mis-reads the scalar on this stack; Copy-with-scale on
                    # ScalarE is the validated path.)
                    scalar.activation(
                        out=w_bfT[:, c * 4:(c + 1) * 4],
                        in_=wrepT_sb[:, c * 4:(c + 1) * 4],
                        func=AF.Copy, scale=winv[:, c:c + 1],
                    )

                # --- main loop: PSUM->SBUF copies + sumsq share ---
                for i in range(NBLK):
                    for up in range(PPB):
                        u = i * PPB + up
                        scalar.wait_ge(petr, u + 1)
                        scalar.activation(
                            out=ftT[u % 2][:], in_=tpb[u % 3][:], func=AF.Copy
                        ).then_inc(actcopy, 1)
                    fb = fbs[i % NB]
                    for j in range(SQ_DVE, SPB):
                        t = i * SPB + j
                        ins = scalar.activation(
                            out=sq_scr[:], in_=fb[:, j * SUB:(j + 1) * SUB],
                            func=AF.Square, accum_out=ss[:, t:t + 1],
                        )
                    ins.then_inc(ablk, 1)

                # --- epilogue (scalar part) ---
                scalar.wait_ge(vdone, NBLK)  # DVE sumsq columns all written
                scalar.activation(out=norm[:], in_=ss[:], func=AF.Sqrt).then_inc(
                    sepi, 1
                )  # sepi=2
                scalar.wait_ge(vepi, 2)  # cos0/cos1 ready
                scalar.activation(out=sq2[:], in_=cos0[:], func=AF.Square)
                scalar.activation(
                    out=sin0[:], in_=sq2[:], func=AF.Sqrt, bias=1.0, scale=-1.0
                )
                scalar.activation(out=sq2[:], in_=cos1[:], func=AF.Square)
                scalar.activation(
                    out=sin1[:], in_=sq2[:], func=AF.Sqrt, bias=1.0, scale=-1.0
                ).then_inc(sepi, 1)  # sepi=3

            @block.vector
            def _(vector):
                # --- preamble: winv + identity cast ---
                vector.wait_ge(sepi, 1)
                vector.tensor_scalar(wnorm[:], wnorm[:], NORM_EPS, None, OP.max)
                vector.reciprocal(winv[:], wnorm[:]).then_inc(vepi, 1)

                # --- main loop: dot harvest + sumsq share ---
                for i in range(NBLK):
                    if i >= 1:
                        vector.wait_ge(pedot, SPB * i)
                        vector.tensor_copy(
                            dd[:].rearrange("p (c t) -> p c t", c=2)
                                [:, :, SPB * (i - 1):SPB * i],
                            d_ps[(i - 1) % DPR][:].rearrange(
                                "p (j c) -> p c j", c=2),
                        )
                    fb = fbs[i % NB]
                    vector.wait_ge(ablk, i + 1)
                    for j in range(SQ_DVE):
                        t = i * SPB + j
                        sl = slice(j * SUB, (j + 1) * SUB)
                        ins = vector.scalar_tensor_tensor(
                            out=tt_scr[:], in0=fb[:, sl], scalar=1.0,
                            in1=fb[:, sl], op0=OP.mult, op1=OP.mult,
                            accum_out=ss[:, t:t + 1],
                        )
                    ins.then_inc(vdone, 1)
                vector.wait_ge(pedot, SPB * NBLK)
                vector.tensor_copy(
                    dd[:].rearrange("p (c t) -> p c t", c=2)
                        [:, :, SPB * (NBLK - 1):SPB * NBLK],
                    d_ps[(NBLK - 1) % DPR][:].rearrange("p (j c) -> p c j", c=2),
                )

                # --- epilogue (vector part) ---
                vector.wait_ge(sepi, 2)  # norm ready
                vector.tensor_scalar(norm[:], norm[:], NORM_EPS, None, OP.max)
                vector.reciprocal(inv[:], norm[:])
                vector.tensor_tensor(cos0[:], dd[:, 0:T], inv[:], OP.mult)
                vector.tensor_scalar(
                    cos0[:], cos0[:], 1.0 - EPS, -1.0 + EPS, OP.min, OP.max
                )
                vector.tensor_tensor(cos1[:], dd[:, T:2 * T], inv[:], OP.mult)
                vector.tensor_scalar(
                    cos1[:], cos1[:], 1.0 - EPS, -1.0 + EPS, OP.min, OP.max
                ).then_inc(vepi, 1)  # vepi=2

                vector.wait_ge(sepi, 3)  # sin0/sin1 ready
                for c, (cosv, sinv) in enumerate(((cos0, sin0), (cos1, sin1))):
                    vector.tensor_scalar(tmp1[:], cosv[:], COS_M, None, OP.mult)
                    vector.tensor_scalar(tmp2[:], sinv[:], SIN_M, None, OP.mult)
                    vector.tensor_tensor(tmp1[:], tmp1[:], tmp2[:], OP.subtract)
                    vector.tensor_tensor(tmp2[:], tmp1[:], cosv[:], OP.subtract)
                    if c == 0:
                        vector.tensor_scalar(
                            oh0[:], lab_t[:], -1.0, 1.0, OP.mult, OP.add
                        )
                        oh = oh0
                    else:
                        oh = lab_t
                    vector.tensor_tensor(tmp2[:], tmp2[:], oh[:], OP.mult)
                    vector.tensor_tensor(tmp2[:], cosv[:], tmp2[:], OP.add)
                    ins = vector.tensor_scalar(
                        out_stage[:, c * T:(c + 1) * T], tmp2[:],
                        S_SCALE, None, OP.mult,
                    )
                ins.then_inc(vepi, 1)  # vepi=3

    return nc


_NC = None


def _get_nc():
    global _NC
    if _NC is None:
        _NC = build_nc()
    return _NC


def _make_in_maps(feat, W, label):
    feat = np.ascontiguousarray(np.asarray(feat, dtype=np.float32))
    W = np.ascontiguousarray(np.asarray(W, dtype=np.float32))
    label = np.asarray(label)
    wr = np.ascontiguousarray(np.tile(W.reshape(1, C * D), (128, 1)))
    # wrepT[p, c*4 + m*2 + par] = W[c, 256*m + 2*p + par]
    # (pair-packed layout matching the f32-packed PE transposes)
    wrT = np.ascontiguousarray(
        W.reshape(C, 2, 128, 2).transpose(2, 0, 1, 3).reshape(128, 8)
    )
    ident = np.eye(128, dtype=np.float32)
    in_maps = []
    for core in range(NCORES):
        fs = feat[core * BS:(core + 1) * BS]
        ls = label[core * BS:(core + 1) * BS].astype(np.float32)
        # lab_dev[p, blk*8+j] = label[blk*1024 + p*8 + j]
        ls = ls.reshape(NBLK, 128, SPB).transpose(1, 0, 2).reshape(128, T)
        in_maps.append(
            {"feat": np.ascontiguousarray(fs), "wrep": wr, "wrepT": wrT,
             "lab": np.ascontiguousarray(ls), "identf": ident}
        )
    return in_maps


def _assemble(results):
    outs = []
    for core in range(NCORES):
        o = np.asarray(results[core]["out"])       # [128, C*T]
        o = o.reshape(128, C, NBLK, SPB)            # [p, c, blk, j]
        o = o.transpose(2, 0, 3, 1).reshape(BS, C)  # [blk, p, j, c]
        outs.append(o)
    return np.concatenate(outs, axis=0)


def run(feat, W, label, trace=False, **kw):
    nc = _get_nc()
    in_maps = _make_in_maps(feat, W, label)
    res = run_bass_kernel_spmd(
        nc, in_maps, core_ids=list(range(NCORES)), trace=trace, **kw
    )
    return _assemble(res.results), res


def kernel(feat, W, label):
    out, _ = run(feat, W, label, trace=False)
    return out


# revision 24
# speedup vs baseline: 1.1370x; 1.0214x over previous
"""ArcFace head forward on 8 Trainium2 NeuronCores (Bass, raw blocks).

Math (per batch row b, class c in {0,1}):
    feat_n = feat / max(||feat||, 1e-12)
    W_n    = W / max(||W_row||, 1e-12)
    cos    = clip(feat_n . W_n[c], -1+1e-7, 1-1e-7)
    cos_m  = cos*cos(0.5) - sqrt(1-cos^2)*sin(0.5)   # == cos(arccos(cos)+0.5)
    out    = 64 * (cos if c != label[b] else cos_m)

Distribution: pure data parallel: feat/label sharded along batch over 8
cores, W replicated; forward only, so no collectives.

Per-core pipeline (shard = 16384 rows x 512 f32 = 32 MB):
  - GpSimd SWDGE streams feat in 16 blocks of [128, 4096], converting
    f32 -> bf16 in flight (SDMA cast); 16 KB-contiguous per partition
  - TensorE: per [128,512] sub-tile, 4x PE-transpose (128x128 bf16) into
    PSUM, then 4 accumulating matmuls (feat^T chunk stationary,
    normalized-W^T chunk moving) -> both class dots in PSUM [128,2]
  - ScalarE: copies transposed tiles PSUM->SBUF (matmul stationary must
    come from SBUF) + a share of the row sum-of-squares
    (activation Square + accum_out)
  - VectorE: the other share of sum-of-squares (fused self-mult STT),
    per-block dot harvest from PSUM, and the batched epilogue
    (norms, clip, margin identity, one-hot blend, scale by 64)
Row mapping: batch row b = blk*1024 + p*8 + j lives on partition p,
accumulator column t = blk*8 + j. Host glue only shards/reorders.
"""

import sys
from contextlib import ExitStack

import numpy as np

for _p in ("/opt/trn_rl_repo",):
    if _p not in sys.path:
        sys.path.insert(0, _p)

import concourse.bass as bass
import concourse.mybir as mybir
from concourse.bass_utils import run_bass_kernel_spmd

B, D, C = 131072, 512, 2
NCORES = 8
BS = B // NCORES          # 16384 rows per core
SUB = 512                 # columns per compute sub-tile
SPB = 8                   # sub-tiles per block
BLK_COLS = SUB * SPB      # 4096 (1024 batch rows)
NBLK = BS // (128 * SPB)  # 16
T = BS // 128             # 128 accumulator columns
NB = 4                    # fb ring depth (1 MB bf16 each)
SQ_DVE = 6                # sub-tiles per block whose sumsq runs on VectorE
PAIRS = T // 2            # transpose/copy granularity: 2 sub-tiles per pair
PPB = SPB // 2            # pairs per block (4)

S_SCALE = 64.0
MARGIN = 0.5
EPS = 1e-7
NORM_EPS = 1e-12
COS_M = float(np.cos(MARGIN))
SIN_M = float(np.sin(MARGIN))

F32 = mybir.dt.float32
BF16 = mybir.dt.bfloat16


def build_nc():
    nc = bass.Bass()
    AF = mybir.ActivationFunctionType
    OP = mybir.AluOpType

    feat = nc.declare_dram_parameter("feat", [BS, D], F32, isOutput=False)
    wrep = nc.declare_dram_parameter("wrep", [128, C * D], F32, isOutput=False)
    wrepT = nc.declare_dram_parameter("wrepT", [128, 2 * 4], F32, isOutput=False)
    lab = nc.declare_dram_parameter("lab", [128, T], F32, isOutput=False)
    identf = nc.declare_dram_parameter("identf", [128, 128], F32, isOutput=False)
    out = nc.declare_dram_parameter("out", [128, C * T], F32, isOutput=True)

    # feat[blk*1024 + p*8 + j, d] -> view[blk, p, j*512+d] (16KB/partition)
    feat_v = feat[:].rearrange("(blk p j) d -> blk p (j d)", p=128, j=SPB)

    with ExitStack() as ctx:
        def sb(name, shape, dt):
            return ctx.enter_context(nc.sbuf_tensor(name, shape, dt))

        def psum(name, shape, dt):
            return ctx.enter_context(nc.psum_tensor(name, shape, dt))

        def sem(name):
            return ctx.enter_context(nc.semaphore(name))

        w_f = sb("w_f", [128, C * D], F32)
        wrepT_sb = sb("wrepT_sb", [128, 8], F32)
        w_bfT = sb("w_bfT", [128, 8], BF16)     # [p, c*4+k] = WnT chunk layout
        lab_t = sb("lab_t", [128, T], F32)
        identf_sb = sb("identf_sb", [128, 128], F32)
        ss = sb("ss", [128, T], F32)
        dd = sb("dd", [128, C * T], F32)        # d0 | d1
        fbs = [sb(f"fb{k}", [128, BLK_COLS], BF16) for k in range(NB)]
        ftT = [sb(f"ftT{k}", [128, SUB], F32) for k in range(3)]
        sq_scr = sb("sq_scr", [128, SUB], F32)
        tt_scr = sb("tt_scr", [128, SUB], BF16)
        wss = sb("wss", [128, C], F32)
        wnorm = sb("wnorm", [128, C], F32)
        winv = sb("winv", [128, C], F32)
        norm = sb("norm", [128, T], F32)
        inv = sb("inv", [128, T], F32)
        cos0 = sb("cos0", [128, T], F32)
        cos1 = sb("cos1", [128, T], F32)
        sq2 = sb("sq2", [128, T], F32)
        sin0 = sb("sin0", [128, T], F32)
        sin1 = sb("sin1", [128, T], F32)
        tmp1 = sb("tmp1", [128, T], F32)
        tmp2 = sb("tmp2", [128, T], F32)
        oh0 = sb("oh0", [128, T], F32)
        out_stage = sb("out_stage", [128, C * T], F32)

        tpb = [psum(f"tpb{k}", [128, SUB], F32) for k in range(3)]
        DPR = NB + 1  # d_ps ring: PE block q is gated only through vdone(q-NB+1),
        # which implies dcopy(q-NB-1) done -> ring must exceed NB
        d_ps = [psum(f"dps{k}", [128, 2 * SPB], F32) for k in range(DPR)]

        pre = sem("pre")       # preamble DMAs
        ld = sem("ld")         # feat block loads (16 per block)
        petr = sem("petr")     # PE transposes done for sub-tile t -> t+1
        actcopy = sem("actcopy")  # ACT copy of sub-tile t done -> t+1
        pedot = sem("pedot")   # PE dots done for sub-tile t -> t+1
        ablk = sem("ablk")     # ACT done with block i -> i+1
        vdone = sem("vdone")   # DVE done with block i -> i+1 (gates fb reuse)
        sepi = sem("sepi")     # scalar milestones
        vepi = sem("vepi")     # vector milestones
        outd = sem("outd")

        # strided view of w_bfT: [p, g, c] with c-stride 4; g = qp*2+par
        w_bfT_g = w_bfT[:].rearrange("p (c g) -> p g c", g=4)

        with nc.Block() as block:

            @block.sync
            def _(sync):
                sync.dma_start(out=w_f[:], in_=wrep[:]).then_inc(pre, 16)
                sync.dma_start(out=wrepT_sb[:], in_=wrepT[:]).then_inc(pre, 16)
                sync.dma_start(out=lab_t[:], in_=lab[:]).then_inc(pre, 16)
                sync.dma_start(out=identf_sb[:], in_=identf[:]).then_inc(pre, 16)
                sync.wait_ge(vepi, 3)
                sync.dma_start(out=out[:], in_=out_stage[:]).then_inc(outd, 16)
                sync.wait_ge(outd, 16)

            @block.gpsimd
            def _(gpsimd):
                for i in range(NBLK):
                    if i >= NB:
                        gpsimd.wait_ge(vdone, i - NB + 1)
                    # converting DMA: f32 DRAM -> bf16 SBUF
                    gpsimd.dma_start(
                        out=fbs[i % NB][:], in_=feat_v[i]
                    ).then_inc(ld, 16)

            @block.tensor
            def _(tensor):
                tensor.wait_ge(pre, 64)  # identf_sb ready
                # software-pipelined: transposes run one pair ahead of dots.
                # Transposes operate on the f32 view of fb (bf16 pairs packed
                # per element): 4 transposes move a whole pair (2 sub-tiles).
                LAG = 2  # dots trail transposes by LAG pairs
                for u in range(PAIRS + LAG):
                    if u < PAIRS:
                        i = u // PPB
                        if u % PPB == 0:
                            tensor.wait_ge(ld, 16 * (i + 1))
                        fb32 = fbs[i % NB][:].bitcast(F32)  # [128, 2048]
                        base = (u % PPB) * SUB              # f32 cols per pair
                        for q in range(4):
                            ins = tensor.transpose(
                                tpb[u % 3][:, q * 128:(q + 1) * 128],
                                fb32[:, base + q * 128: base + (q + 1) * 128],
                                identf_sb[:],
                            )
                        ins.then_inc(petr, 1)
                    if u >= LAG:
                        ud = u - LAG
                        tensor.wait_ge(actcopy, ud + 1)
                        # bf16 view of the packed transposed pair:
                        # [p, q, b, par] with q = s*2+m, D = 256*m + 2*dp + par
                        vw = ftT[ud % 3][:].bitcast(BF16).rearrange(
                            "p (q b par) -> p q par b", q=4, par=2)
                        for s in range(2):
                            td = 2 * ud + s
                            i_d, j_d = td // SPB, td % SPB
                            for g in range(4):
                                m, par = g // 2, g % 2
                                ins = tensor.matmul(
                                    d_ps[i_d % DPR][:, 2 * j_d:2 * j_d + 2],
                                    vw[:, s * 2 + m, par, :],
                                    w_bfT_g[:, g, :],
                                    start=(g == 0), stop=(g == 3),
                                )
                        ins.then_inc(pedot, 1)

            @block.scalar
            def _(scalar):
                # --- W norms ---
                scalar.wait_ge(pre, 64)
                for c in range(C):
                    scalar.activation(
                        out=sq_scr[:], in_=w_f[:, c * D:(c + 1) * D],
                        func=AF.Square, accum_out=wss[:, c:c + 1],
                    )
                scalar.activation(out=wnorm[:], in_=wss[:], func=AF.Sqrt).then_inc(
                    sepi, 1
                )  # sepi=1
                scalar.wait_ge(vepi, 1)  # winv ready
                for c in range(C):
                    # w_bfT = wrepT * (1/||W_c||), cast to bf16.
                    # (tensor_scalar with an AP scalar = TensorScalarPtr
                    # mis-reads the scalar on this stack; Copy-with-scale on
                    # ScalarE is the validated path.)
                    scalar.activation(
                        out=w_bfT[:, c * 4:(c + 1) * 4],
                        in_=wrepT_sb[:, c * 4:(c + 1) * 4],
                        func=AF.Copy, scale=winv[:, c:c + 1],
                    )

                # --- main loop: PSUM->SBUF copies + sumsq share ---
                for i in range(NBLK):
                    for up in range(PPB):
                        u = i * PPB + up
                        scalar.wait_ge(petr, u + 1)
                        scalar.activation(
                            out=ftT[u % 3][:], in_=tpb[u % 3][:], func=AF.Copy
                        ).then_inc(actcopy, 1)
                    fb = fbs[i % NB]
                    for j in range(SQ_DVE, SPB):
                        t = i * SPB + j
                        ins = scalar.activation(
                            out=sq_scr[:], in_=fb[:, j * SUB:(j + 1) * SUB],
                            func=AF.Square, accum_out=ss[:, t:t + 1],
                        )
                    ins.then_inc(ablk, 1)

                # --- epilogue (scalar part) ---
                scalar.wait_ge(vdone, NBLK)  # DVE sumsq columns all written
                scalar.activation(out=norm[:], in_=ss[:], func=AF.Sqrt).then_inc(
                    sepi, 1
                )  # sepi=2
                scalar.wait_ge(vepi, 2)  # cos0/cos1 ready
                scalar.activation(out=sq2[:], in_=cos0[:], func=AF.Square)
                scalar.activation(
                    out=sin0[:], in_=sq2[:], func=AF.Sqrt, bias=1.0, scale=-1.0
                )
                scalar.activation(out=sq2[:], in_=cos1[:], func=AF.Square)
                scalar.activation(
                    out=sin1[:], in_=sq2[:], func=AF.Sqrt, bias=1.0, scale=-1.0
                ).then_inc(sepi, 1)  # sepi=3

            @block.vector
            def _(vector):
                # --- preamble: winv + identity cast ---
                vector.wait_ge(sepi, 1)
                vector.tensor_scalar(wnorm[:], wnorm[:], NORM_EPS, None, OP.max)
                vector.reciprocal(winv[:], wnorm[:]).then_inc(vepi, 1)

                # --- main loop: dot harvest + sumsq share ---
                for i in range(NBLK):
                    if i >= 1:
                        vector.wait_ge(pedot, PPB * i)
                        vector.tensor_copy(
                            dd[:].rearrange("p (c t) -> p c t", c=2)
                                [:, :, SPB * (i - 1):SPB * i],
                            d_ps[(i - 1) % DPR][:].rearrange(
                                "p (j c) -> p c j", c=2),
                        )
                    fb = fbs[i % NB]
                    vector.wait_ge(ablk, i + 1)
                    for j in range(SQ_DVE):
                        t = i * SPB + j
                        sl = slice(j * SUB, (j + 1) * SUB)
                        ins = vector.scalar_tensor_tensor(
                            out=tt_scr[:], in0=fb[:, sl], scalar=1.0,
                            in1=fb[:, sl], op0=OP.mult, op1=OP.mult,
                            accum_out=ss[:, t:t + 1],
                        )
                    ins.then_inc(vdone, 1)
                vector.wait_ge(pedot, PPB * NBLK)
                vector.tensor_copy(
                    dd[:].rearrange("p (c t) -> p c t", c=2)
                        [:, :, SPB * (NBLK - 1):SPB * NBLK],
                    d_ps[(NBLK - 1) % DPR][:].rearrange("p (j c) -> p c j", c=2),
                )

                # --- epilogue (vector part) ---
                vector.wait_ge(sepi, 2)  # norm ready
                vector.tensor_scalar(norm[:], norm[:], NORM_EPS, None, OP.max)
                vector.reciprocal(inv[:], norm[:])
                vector.tensor_tensor(cos0[:], dd[:, 0:T], inv[:], OP.mult)
                vector.tensor_scalar(
                    cos0[:], cos0[:], 1.0 - EPS, -1.0 + EPS, OP.min, OP.max
                )
                vector.tensor_tensor(cos1[:], dd[:, T:2 * T], inv[:], OP.mult)
                vector.tensor_scalar(
                    cos1[:], cos1[:], 1.0 - EPS, -1.0 + EPS, OP.min, OP.max
                ).then_inc(vepi, 1)  # vepi=2

                vector.wait_ge(sepi, 3)  # sin0/sin1 ready
                for c, (cosv, sinv) in enumerate(((cos0, sin0), (cos1, sin1))):
                    vector.tensor_scalar(tmp1[:], cosv[:], COS_M, None, OP.mult)
                    vector.tensor_scalar(tmp2[:], sinv[:], SIN_M, None, OP.mult)
                    vector.tensor_tensor(tmp1[:], tmp1[:], tmp2[:], OP.subtract)
                    vector.tensor_tensor(tmp2[:], tmp1[:], cosv[:], OP.subtract)
                    if c == 0:
                        vector.tensor_scalar(
                            oh0[:], lab_t[:], -1.0, 1.0, OP.mult, OP.add
                        )
                        oh = oh0
                    else:
                        oh = lab_t
                    vector.tensor_tensor(tmp2[:], tmp2[:], oh[:], OP.mult)
                    vector.tensor_tensor(tmp2[:], cosv[:], tmp2[:], OP.add)
                    ins = vector.tensor_scalar(
                        out_stage[:, c * T:(c + 1) * T], tmp2[:],
                        S_SCALE, None, OP.mult,
                    )
                ins.then_inc(vepi, 1)  # vepi=3

    return nc


_NC = None


def _get_nc():
    global _NC
    if _NC is None:
        _NC = build_nc()
    return _NC


def _make_in_maps(feat, W, label):
    feat = np.ascontiguousarray(np.asarray(feat, dtype=np.float32))
    W = np.ascontiguousarray(np.asarray(W, dtype=np.float32))
    label = np.asarray(label)
    wr = np.ascontiguousarray(np.tile(W.reshape(1, C * D), (128, 1)))
    # wrepT[p, c*4 + m*2 + par] = W[c, 256*m + 2*p + par]
    # (pair-packed layout matching the f32-packed PE transposes)
    wrT = np.ascontiguousarray(
        W.reshape(C, 2, 128, 2).transpose(2, 0, 1, 3).reshape(128, 8)
    )
    ident = np.eye(128, dtype=np.float32)
    in_maps = []
    for core in range(NCORES):
        fs = feat[core * BS:(core + 1) * BS]
        ls = label[core * BS:(core + 1) * BS].astype(np.float32)
        # lab_dev[p, blk*8+j] = label[blk*1024 + p*8 + j]
        ls = ls.reshape(NBLK, 128, SPB).transpose(1, 0, 2).reshape(128, T)
        in_maps.append(
            {"feat": np.ascontiguousarray(fs), "wrep": wr, "wrepT": wrT,
             "lab": np.ascontiguousarray(ls), "identf": ident}
        )
    return in_maps


def _assemble(results):
    outs = []
    for core in range(NCORES):
        o = np.asarray(results[core]["out"])       # [128, C*T]
        o = o.reshape(128, C, NBLK, SPB)            # [p, c, blk, j]
        o = o.transpose(2, 0, 3, 1).reshape(BS, C)  # [blk, p, j, c]
        outs.append(o)
    return np.concatenate(outs, axis=0)


def run(feat, W, label, trace=False, **kw):
    nc = _get_nc()
    in_maps = _make_in_maps(feat, W, label)
    res = run_bass_kernel_spmd(
        nc, in_maps, core_ids=list(range(NCORES)), trace=trace, **kw
    )
    return _assemble(res.results), res


def kernel(feat, W, label):
    out, _ = run(feat, W, label, trace=False)
    return out
